# revision 3
# baseline (speedup 1.0000x reference)
"""Trainium2 Bass kernel for nn_MultiHeadAttention_59227599012491.

Reference computation (per batch b):
    xf = x[b].reshape(S, 256)
    q  = softplus(xf @ Wq.T + bq);  k = softplus(xf @ Wk.T + bk)
    v  = xf @ Wv.T + bv
    out = ((q @ k.T) @ v) @ Wo.T + bo          (no softmax!)

No softmax -> attention is associative:
    out = q @ M + bo,   M = (k.T v) Wo.T-form folded to [256, 256]
and v itself is LINEAR in x, so v is never materialized:
    k.T v = k.T (x Wv.T + 1 bv.T) = (k.T x) Wv.T + s_k bv.T
    HT[c,e] = sum_s x[s,c] k[s,e]        (lhsT = x_nat chunk, rhs = k tile)
    GT[d,e] = sum_c WvT[c,d] HT[c,e]
    M[e,do] = sum_d GT[d,e] WoT[d,do] + sum_s' Kbar[s',e] u[do]
where Kbar[s',e] = sum_t k[128t+s', e] (partition-folded k sum, accumulated
on the otherwise-idle GPSIMD engine) and u = Wo bv is host-precomputed;
the bv rank-1 correction rides the M matmul as one extra 128-contract
channel against a host-replicated u tile.

vs the v1 kernel this removes the entire v path: the kv projection
matmuls halve (PE -7us) and the PSUM->SBUF eviction volume halves
(DVE 22us -> ~12us); the extra x_nat load (+2MB, natural layout, no
host transpose) rides free DMA bandwidth. ACT (softplus = Exp+Ln,
3.15M elems) becomes the pacing engine.

Sharding: B=4 batches x 2 query-halves -> 8 cores, no collectives
(an NRT AllReduce was measured at ~17us fixed rendezvous - more than
the k/H dedup saves). k/H/G/M are recomputed per pair; only q/out rows
split. x_nat needs no per-core rotation (it is only contracted over
all of s).

Layouts (PE computes out = lhsT.T @ rhs, contracting partition dim):
    xbT  [256, 4096]  x[b] transposed on host (queries first SQ cols)
    xnat [4096, 256]  x[b] natural row-major  (shared by the pair)
    k    [4096, 256]  32 tiles; psum + bk on DVE, softplus on ACT
                      (batch-size ramp 2,4,8,8 then 4,2,2,2)
    qT   [256, 2048]  lhsT = Wq chunk; softplus fused w/ per-part bias
    outT [256, 2048]  lhsT = M block; bo per-partition on DVE; fp16
                      dump, host un-transposes

Scheduling facts carried over from v1 (all measured): single sync-ring
FIFO position is the DMA priority mechanism; biasc right after x0;
PSUM accumulation groups must not share a bank (H chunks get a full
bank each); activation tables steered so the one table holding Exp AND
Ln is loaded exactly once.
"""

import numpy as np

S = 4096
SQ = 2048  # query rows per core
D = 256
P = 128
IT = D // P  # 2 input-dim tiles
DT = D // P  # 2 d-model tiles
NS = S // P  # 32 sequence tiles
BLK = 512  # free-dim block for qT
N_CORES = 8

MM_DTYPE_NAME = "float16"

_CACHE = {}


def _patched_act_tables(orig_fn):
    def patched(arch):
        tabs = orig_fn(arch)
        return {
            name: (s if name == "natural_log_exp_and_others" else set())
            for name, s in tabs.items()
        }

    return patched


def _build_nc():
    import concourse.bacc as bacc
    import concourse.mybir as mybir
    import concourse.tile as tile

    FP = mybir.dt.float32
    FR = getattr(mybir.dt, MM_DTYPE_NAME)
    AF = mybir.ActivationFunctionType
    ADD = mybir.AluOpType.add

    nc = bacc.Bacc("TRN2", target_bir_lowering=False, debug=False, num_devices=1)

    xbT_d = nc.declare_dram_parameter("xbT", [D, S], FR, isOutput=False)
    xnat_d = nc.declare_dram_parameter("xnat", [S, D], FR, isOutput=False)
    # WkT it-blocks side by side: [128, (it0 256 | it1 256)]
    wkp_d = nc.declare_dram_parameter("wkp", [P, IT * D], FR, isOutput=False)
    # wq it-blocks | wo dt-blocks as in v1: [128, 1024]
    wqop_d = nc.declare_dram_parameter("wqop", [P, 1024], FR, isOutput=False)
    # WvT c-blocks [128, 512] then u = Wo bv replicated [128, 256]
    wvu_d = nc.declare_dram_parameter("wvu", [P, IT * D + D], FR, isOutput=False)
    # biases: cols 0:4 = bqT|boT dt-chunks, 4:260 = bk row replicated
    bias_d = nc.declare_dram_parameter("biasc", [P, 4 + D], FP, isOutput=False)
    outp_d = nc.declare_dram_parameter("outp", [P, 2 * SQ], FR, isOutput=True)

    def mm(psum, lhsT, rhs, start, stop):
        nc.tensor.matmul(psum, lhsT, rhs, start=start, stop=stop)

    with tile.TileContext(nc) as tc:
        with (
            tc.tile_pool(name="w", bufs=1) as wpool,
            tc.tile_pool(name="big", bufs=1) as big,
            tc.tile_pool(name="tmp", bufs=4) as tpool,
            tc.tile_pool(name="psQ", bufs=2, space="PSUM") as psQ,
            tc.tile_pool(name="psK", bufs=3, space="PSUM") as psK,
            tc.tile_pool(name="psH", bufs=1, space="PSUM") as psH,
        ):
            wk_sb = wpool.tile([P, IT * D], FR, tag="wk")
            wqo_sb = wpool.tile([P, 1024], FR, tag="wqo")
            wvu_sb = wpool.tile([P, IT * D + D], FR, tag="wvu")
            xbT_sb = big.tile([P, IT, S], FR, tag="xbT")
            xnat_sb = big.tile([P, NS, D], FR, tag="xnat")
            biasc = wpool.tile([P, 4 + D], FP, tag="biasc")
            bias_sb = biasc[:, 0:4]
            bk_bc = biasc[:, 4 : 4 + D]

            # --- input DMAs, sync-ring FIFO order is priority ---
            nc.sync.dma_start(wk_sb[:, :], wkp_d.ap()[:, :])
            for it in range(IT):
                nc.sync.dma_start(
                    xbT_sb[:, it, 0:1024], xbT_d.ap()[it * P : (it + 1) * P, 0:1024]
                )
            nc.sync.dma_start(biasc[:, :], bias_d.ap()[:, :])
            nc.sync.dma_start(wqo_sb[:, :], wqop_d.ap()[:, :])
            for it in range(IT):
                nc.sync.dma_start(
                    xbT_sb[:, it, 1024:2048], xbT_d.ap()[it * P : (it + 1) * P, 1024:2048]
                )
            # x natural-layout tiles: DRAM rows (128 t + p) -> [p, t, c]
            xn_ap = xnat_d.ap().rearrange("(t p) c -> p t c", p=P)
            nc.sync.dma_start(xnat_sb[:, 0:8, :], xn_ap[:, 0:8, :])
            for it in range(IT):
                nc.sync.dma_start(
                    xbT_sb[:, it, 2048:3072], xbT_d.ap()[it * P : (it + 1) * P, 2048:3072]
                )
            nc.sync.dma_start(xnat_sb[:, 8:16, :], xn_ap[:, 8:16, :])
            for it in range(IT):
                nc.sync.dma_start(
                    xbT_sb[:, it, 3072:4096], xbT_d.ap()[it * P : (it + 1) * P, 3072:4096]
                )
            nc.sync.dma_start(xnat_sb[:, 16:24, :], xn_ap[:, 16:24, :])
            nc.sync.dma_start(xnat_sb[:, 24:32, :], xn_ap[:, 24:32, :])
            nc.sync.dma_start(wvu_sb[:, :], wvu_d.ap()[:, :])

            k_sb = big.tile([P, NS, D], FR, tag="k")
            qT_sb = big.tile([P, DT, SQ], FR, tag="qT")
            outT_sb = big.tile([P, DT, SQ], FR, tag="outT")
            HT_sb = wpool.tile([P, IT, D], FR, tag="HT")
            GT_sb = wpool.tile([P, DT, D], FR, tag="GT")
            M_sb = wpool.tile([P, DT, D], FR, tag="M")
            Kbar = wpool.tile([P, D], FR, tag="Kbar")

            # persistent H accumulators: one full PSUM bank per c-chunk so the
            # two 32-tile accumulation groups never share a bank
            psH0 = psH.tile([P, 512], FP, tag="psH0")
            psH1 = psH.tile([P, 512], FP, tag="psH1")
            psHc = [psH0, psH1]

            # batch-size ramp: small first so the saturated ACT engine starts
            # ASAP, small last so the final softplus->H->G->M->out chain stays
            # short
            SPB = {1: 2, 5: 4, 13: 8, 21: 8, 25: 4, 27: 2, 29: 2, 31: 2}

            for t in range(NS):
                ts = slice(t * P, (t + 1) * P)
                ps = psK.tile([P, 512], FP, tag="psK")
                for it in range(IT):
                    mm(
                        ps[:, 0:D],
                        xbT_sb[:, it, ts],
                        wk_sb[:, it * D : (it + 1) * D],
                        it == 0,
                        it == IT - 1,
                    )
                nc.vector.tensor_tensor(k_sb[:, t, :], ps[:, 0:D], bk_bc, op=ADD)
                bsz = SPB.get(t, 0)
                if bsz:
                    tt = slice(t - bsz + 1, t + 1)
                    tmp = tpool.tile([P, bsz, D], FP, tag=f"tmpk{bsz}")
                    nc.scalar.activation(tmp[:, :, :], k_sb[:, tt, :], AF.Exp)
                    nc.scalar.activation(k_sb[:, tt, :], tmp[:, :, :], AF.Ln, bias=1.0)
                    for t2 in range(t - bsz + 1, t + 1):
                        # H accumulation: lhsT = x_nat c-chunk, rhs = k tile
                        for c in range(IT):
                            mm(
                                psHc[c][:, 0:D],
                                xnat_sb[:, t2, c * P : (c + 1) * P],
                                k_sb[:, t2, :],
                                t2 == 0,
                                t2 == NS - 1,
                            )
                        # Kbar += k tile on the idle GPSIMD engine
                        if t2 == 0:
                            nc.gpsimd.tensor_copy(Kbar[:, :], k_sb[:, 0, :])
                        else:
                            nc.gpsimd.tensor_tensor(
                                Kbar[:, :], Kbar[:, :], k_sb[:, t2, :], op=ADD
                            )

            # q path: lhsT = Wq chunk (stationary), softplus fused from PSUM
            for dt in range(DT):
                for half in range(SQ // (2 * BLK)):
                    tmp = tpool.tile([P, 2, BLK], FP, tag="tmpq")
                    for c in range(2):
                        blk = 2 * half + c
                        ss = slice(blk * BLK, (blk + 1) * BLK)
                        ps = psQ.tile([P, BLK], FP, tag="psQ")
                        for it in range(IT):
                            mm(
                                ps[:, :],
                                wqo_sb[:, it * D + dt * P : it * D + (dt + 1) * P],
                                xbT_sb[:, it, ss],
                                it == 0,
                                it == IT - 1,
                            )
                        nc.scalar.activation(
                            tmp[:, c, :], ps[:, :], AF.Exp, bias=bias_sb[:, dt : dt + 1]
                        )
                    nc.scalar.activation(
                        qT_sb[:, dt, 2 * half * BLK : 2 * (half + 1) * BLK],
                        tmp[:, :, :].rearrange("p a b -> p (a b)"),
                        AF.Ln,
                        bias=1.0,
                    )

            # evict H, then GT = WvT-chunks x HT
            for c in range(IT):
                nc.vector.tensor_copy(HT_sb[:, c, :], psHc[c][:, 0:D])
            for dc in range(DT):
                ps = psK.tile([P, 512], FP, tag="psK")
                for c in range(IT):
                    mm(
                        ps[:, 0:D],
                        wvu_sb[:, c * D + dc * P : c * D + (dc + 1) * P],
                        HT_sb[:, c, :],
                        c == 0,
                        c == IT - 1,
                    )
                nc.vector.tensor_copy(GT_sb[:, dc, :], ps[:, 0:D])

            # M[e,do] = sum_d GT[d,e] WoT[d,do] + sum_s' Kbar[s',e] u[do]
            for et in range(DT):
                es = slice(et * P, (et + 1) * P)
                ps = psK.tile([P, 512], FP, tag="psK")
                for dt in range(DT):
                    mm(
                        ps[:, 0:D],
                        GT_sb[:, dt, es],
                        wqo_sb[:, 512 + dt * D : 512 + (dt + 1) * D],
                        dt == 0,
                        False,
                    )
                mm(ps[:, 0:D], Kbar[:, es], wvu_sb[:, IT * D :], False, True)
                nc.vector.tensor_copy(M_sb[:, et, :], ps[:, 0:D])

            # outT[do, s] = M^T q^T + bo: lhsT = M block (stationary), bo is
            # per-partition on the DVE, fp16 transposed dump
            for dot in range(DT):
                for blk in range(SQ // BLK):
                    ss = slice(blk * BLK, (blk + 1) * BLK)
                    ps = psQ.tile([P, BLK], FP, tag="psQ")
                    for et in range(DT):
                        mm(
                            ps[:, :],
                            M_sb[:, et, dot * P : (dot + 1) * P],
                            qT_sb[:, et, ss],
                            et == 0,
                            et == DT - 1,
                        )
                    if dot == DT - 1 and blk == SQ // BLK - 1:
                        # very last chunk: split the eviction across the idle
                        # ACT and DVE so the serial tail halves
                        nc.scalar.activation(
                            outT_sb[:, dot, blk * BLK : blk * BLK + 256],
                            ps[:, 0:256],
                            AF.Identity,
                            bias=bias_sb[:, 2 + dot : 3 + dot],
                        )
                        nc.vector.tensor_scalar_add(
                            outT_sb[:, dot, blk * BLK + 256 : (blk + 1) * BLK],
                            ps[:, 256:512],
                            bias_sb[:, 2 + dot : 3 + dot],
                        )
                    else:
                        nc.vector.tensor_scalar_add(
                            outT_sb[:, dot, ss], ps[:, :], bias_sb[:, 2 + dot : 3 + dot]
                        )
                    if dot == DT - 1 and blk >= SQ // BLK - 2:
                        # ship the last two chunks individually so only one
                        # chunk's descriptors remain after the final eviction
                        off = dot * SQ + blk * BLK
                        src_ap = outT_sb[:, dot, blk * BLK : (blk + 1) * BLK]
                        if blk == SQ // BLK - 1:
                            nc.sync.dma_start(
                                outp_d.ap()[0:64, off : off + BLK], src_ap[0:64, :]
                            )
                            nc.scalar.dma_start(
                                outp_d.ap()[64:P, off : off + BLK], src_ap[64:P, :]
                            )
                        else:
                            nc.sync.dma_start(outp_d.ap()[:, off : off + BLK], src_ap)
                    elif blk % 2 == 1:
                        off = dot * SQ + (blk - 1) * BLK
                        src_ap = outT_sb[:, dot, (blk - 1) * BLK : (blk + 1) * BLK]
                        nc.sync.dma_start(outp_d.ap()[:, off : off + 2 * BLK], src_ap)

    import concourse.hw_specs as hw_specs

    orig = bacc.get_activation_tables
    bacc.get_activation_tables = _patched_act_tables(hw_specs.get_activation_tables)
    try:
        nc.compile()
    finally:
        bacc.get_activation_tables = orig
    return nc


def _get_nc():
    nc = _CACHE.get("nc")
    if nc is None:
        nc = _build_nc()
        _CACHE["nc"] = nc
    return nc


def make_in_maps(x, Wq, bq, Wk, bk, Wv, bv, Wo, bo):
    B = x.shape[0]
    mmnp = np.float16
    xf = np.asarray(x, dtype=np.float32).reshape(B, S, D)
    xfT = np.ascontiguousarray(xf.transpose(0, 2, 1).astype(mmnp))
    xnat = np.ascontiguousarray(xf.astype(mmnp))
    wk2 = np.asarray(Wk, mmnp).T
    wkp = np.ascontiguousarray(np.hstack([wk2[0:P], wk2[P:D]]))  # [128, 512]
    wq2 = np.asarray(Wq, mmnp).T
    wo2 = np.asarray(Wo, mmnp).T
    wqop = np.ascontiguousarray(
        np.hstack([wq2[0:P], wq2[P:D], wo2[0:P], wo2[P:D]])
    )  # [128, (it0 wq|it1 wq|dt0 wo|dt1 wo)]
    wv2 = np.asarray(Wv, mmnp).T  # WvT [c, d]
    u = (np.asarray(Wo, np.float64) @ np.asarray(bv, np.float64)).astype(mmnp)
    wvu = np.ascontiguousarray(
        np.hstack([wv2[0:P], wv2[P:D], np.tile(u, (P, 1))])
    )  # [128, (c0 wv|c1 wv|u tiled)]
    biasc = np.ascontiguousarray(
        np.hstack(
            [
                np.stack(
                    [
                        np.asarray(bq, np.float32)[0:P],
                        np.asarray(bq, np.float32)[P:D],
                        np.asarray(bo, np.float32)[0:P],
                        np.asarray(bo, np.float32)[P:D],
                    ],
                    axis=1,
                ),
                np.tile(np.asarray(bk, np.float32), (P, 1)),
            ]
        )
    )
    shared = {
        "wkp": wkp,
        "wqop": wqop,
        "wvu": wvu,
        "biasc": biasc,
    }
    in_maps = []
    for c in range(N_CORES):
        b, h = divmod(c, 2)
        xT = xfT[b]
        xn = xnat[b]
        if h == 1:
            # query rows first; xnat must match xbT's s-order since the H
            # matmul pairs x_nat tile t with k tile t row-for-row
            xT = np.concatenate([xT[:, SQ:], xT[:, :SQ]], axis=1)
            xn = np.concatenate([xn[SQ:], xn[:SQ]], axis=0)
        in_maps.append(
            {"xbT": np.ascontiguousarray(xT), "xnat": np.ascontiguousarray(xn), **shared}
        )
    return in_maps


def assemble_out(results, x_shape):
    B, S_, H, W = x_shape
    out = np.empty((B, S_, D), np.float32)
    for c in range(N_CORES):
        b, h = divmod(c, 2)
        outp = results[c]["outp"]  # [128, 2*SQ] fp16: [p, dot*SQ + s]
        v = outp.reshape(P, DT, SQ).astype(np.float32)
        out[b, h * SQ : (h + 1) * SQ] = v.transpose(2, 1, 0).reshape(SQ, D)
    return out.reshape(B, S_, H, W)


def kernel(x, Wq, bq, Wk, bk, Wv, bv, Wo, bo, _trace=False):
    from concourse.bass_utils import run_bass_kernel_spmd

    nc = _get_nc()
    in_maps = make_in_maps(x, Wq, bq, Wk, bk, Wv, bv, Wo, bo)
    res = run_bass_kernel_spmd(nc, in_maps, list(range(N_CORES)), trace=_trace)
    out = assemble_out(res.results, x.shape)
    if _trace:
        _CACHE["last_result"] = res
    return out
